# revision 19
# baseline (speedup 1.0000x reference)
"""Multi-head attention (B=4, N=1024, C=1024, H=16, D=64) on 8 Trainium2 cores.

Sharding: query-parallel, no collectives. Core i handles batch b = i//2 and
query rows (i%2)*512..+512 (the host rolls x[b] so each core's query rows come
first; key order is irrelevant to softmax). Each core computes k/v for its
whole batch, attention for its 512 query rows over all 16 heads, and the
output projection for those rows. The host concatenates the 8 row-slices --
softmax rows are independent, so no cross-core reduction is needed.

Matmuls run in fp16 (1 PE cycle/row vs 1.5 for fp32r, FWL weight loads,
half-size DMA, 10-bit mantissa). Accumulation is always fp32 in PSUM. exp is
computed as exp(s/8 - 12*ln2) so unnormalized attention outputs stay in fp16
range; the 2^-12 factor cancels in the softmax normalization.

Per-core pipeline (x^T is prepared on the host -- input marshalling):
  1. v = x @ w_v -> [m, h, d] with an appended ones column, key mask folded in.
  2. Per head pair t: k^T/q^T column projections for pair t only, then
     S^T = k^T.T @ q^T (K=64 row-tiles, the pair alternates PE row groups
     0/64 so matmuls overlap) -> exp on ACT -> out^T (unnormalized) and the
     softmax denominator via the ones column -> stage to SBUF.
     Attention of pair t overlaps projections of pair t+1 on the PE.
  3. Per head quad: one DVE reciprocal of the 4 denominators, gpsimd
     partition-broadcast, DVE in-place normalize of out^T.
  4. y = o^T.T @ w_out + bias (K=1 ones-row matmul), DMA out.
"""

import os

import numpy as np

import concourse.bacc as bacc
import concourse.mybir as mybir
import concourse.tile as tile
from concourse.bass_utils import run_bass_kernel_spmd

F32 = mybir.dt.float32
F16 = mybir.dt.float16

B, N, C = 4, 1024, 1024
H, D = 16, 64
NQ = 512          # query rows per core
P = 128
MO = N // P       # 8 key m-tiles
CO = C // P       # 8 contraction tiles
EO = C // P       # 8 e-tiles for q (and separately k)
NO = NQ // P      # 4 output row tiles
ATT_SCALE = D ** -0.5
EXP_BIAS = float(-12.0 * np.log(2.0))  # keep out^T in fp16 range
N_CORES = 8


def build_nc():
    nc = bacc.Bacc()
    xbT = nc.declare_dram_parameter("xbT", [C, N], F16, isOutput=False)
    maskb = nc.declare_dram_parameter("maskb", [N], F32, isOutput=False)
    w_qkv = nc.declare_dram_parameter("w_qkv", [C, 3 * H * D], F16, isOutput=False)
    w_out = nc.declare_dram_parameter("w_out", [H * D, C], F16, isOutput=False)
    b_out = nc.declare_dram_parameter("b_out", [C], F16, isOutput=False)
    y = nc.declare_dram_parameter("y", [NQ, C], F32, isOutput=True)

    xbT_t = xbT.rearrange("(co p) m -> p co m", p=P)
    wqkv_t = w_qkv.rearrange("(co p) e -> p co e", p=P)
    wout_t = w_out.rearrange("(eo p) c -> p eo c", p=P)
    y_t = y.rearrange("(no p) c -> p no c", p=P)

    with tile.TileContext(nc) as tc:
        with tc.tile_pool(name="consts", bufs=1) as consts, \
             tc.tile_pool(name="persist", bufs=1) as persist:
            # ---- constants ----
            ones_row = consts.tile([1, P], F16)      # K=1 bias matmul lhsT
            nc.vector.memset(ones_row[:], 1.0)
            onesH = consts.tile([P, H], F16)
            nc.vector.memset(onesH[:], 1.0)
            ones_q = consts.tile([97, 64], F16)
            nc.vector.memset(ones_q[:], 1.0)
            mask_sb = consts.tile([P, MO], F32)
            nc.sync.dma_start(mask_sb[:], maskb.rearrange("(o p) -> p o", p=P))
            ebias = consts.tile([P, 1], F32)
            nc.vector.memset(ebias[:], EXP_BIAS)
            bias_sb = consts.tile([1, C], F16)
            nc.sync.dma_start(bias_sb[:], b_out[None, :])

            # ---- persistent tensors ----
            qT = persist.tile([P, EO, NQ], F16)          # q^T: [e, n]
            kT = persist.tile([P, EO, N], F16)           # k^T: [e, m]
            v_sb = persist.tile([P, MO, H, D + 1], F16)  # v + ones col
            o_sb = persist.tile([P, EO, NQ], F16)        # out^T: [e, n]
            wo = persist.tile([P, EO, C], F16)           # w_out staged early

            with tc.tile_pool(name="xT_pool", bufs=1) as xT_pool, \
                 tc.tile_pool(name="w_pool", bufs=4) as w_pool, \
                 tc.tile_pool(name="E_pool", bufs=3) as E_pool, \
                 tc.tile_pool(name="sm_pool", bufs=2) as sm_pool, \
                 tc.tile_pool(name="ou_pool", bufs=5) as ou_pool, \
                 tc.tile_pool(name="ps_proj", bufs=2, space="PSUM") as ps_proj, \
                 tc.tile_pool(name="ps_s", bufs=2, space="PSUM") as ps_s, \
                 tc.tile_pool(name="ps_av", bufs=2, space="PSUM") as ps_av:
                xT = xT_pool.tile([P, CO, N], F16)       # x^T: [c, m]
                nc.sync.dma_start(xT[:, 0, :], xbT_t[:, 0, :])

                def kq_proj(t, wk=None, wq=None):
                    # k^T columns for pair t (k block starts at e = C)
                    if wk is None:
                        wk = w_pool.tile([P, CO, P], F16, tag="wqk",
                                         name=f"wk{t}")
                        nc.sync.dma_start(
                            wk[:], wqkv_t[:, :, C + t * P:C + (t + 1) * P])
                    for half in range(2):
                        pk = ps_proj.tile([P, NQ], F32, tag="pp", name=f"pk{t}_{half}")
                        for co in range(CO):
                            nc.tensor.matmul(
                                pk[:], wk[:, co, :],
                                xT[:, co, half * NQ:(half + 1) * NQ],
                                start=(co == 0), stop=(co == CO - 1))
                        nc.vector.tensor_copy(
                            kT[:, t, half * NQ:(half + 1) * NQ], pk[:])
                    if wq is None:
                        wq = w_pool.tile([P, CO, P], F16, tag="wqk",
                                         name=f"wq{t}")
                        nc.sync.dma_start(
                            wq[:], wqkv_t[:, :, t * P:(t + 1) * P])
                    pq = ps_proj.tile([P, NQ], F32, tag="pp", name=f"pq{t}")
                    for co in range(CO):
                        nc.tensor.matmul(
                            pq[:], wq[:, co, :], xT[:, co, 0:NQ],
                            start=(co == 0), stop=(co == CO - 1))
                    nc.vector.tensor_copy(qT[:, t, :], pq[:])

                def s_exp(t):
                    # S^T + exp: one [128,1024] psum per mo holds both heads of
                    # the pair (row groups 0/64 -> the two matmuls overlap)
                    E_pair = E_pool.tile([P, MO, 2, NQ], F16, tag="E",
                                         name=f"E{t}")
                    for mo in range(MO):
                        pss = ps_s.tile([P, 2 * NQ], F32, tag="ps_s",
                                        name=f"pss{t}_{mo}")
                        for j in range(2):
                            pb = 64 * j
                            nc.tensor.matmul(
                                pss[:, j * NQ:(j + 1) * NQ],
                                kT[pb:pb + 64, t, mo * P:(mo + 1) * P],
                                qT[pb:pb + 64, t, :],
                                start=True, stop=True)
                        nc.scalar.activation(
                            E_pair[:, mo, :, :], pss[:],
                            mybir.ActivationFunctionType.Exp,
                            bias=ebias[:], scale=ATT_SCALE)
                    return E_pair

                norm_state = {}

                def av_norm(t, E_pair):
                    # AV + stage unnormalized out^T and denominator
                    den_p = sm_pool.tile([33, NQ], F32, tag="den",
                                         name=f"den{t}")
                    nc.vector.memset(den_p[:], 1.0)
                    o_uns = []
                    for j in range(2):
                        h = 2 * t + j
                        pb = 64 * j
                        pav = ps_av.tile([P, NQ], F32, tag="ps_av",
                                         name=f"pav{h}")
                        for mo in range(MO):
                            nc.tensor.matmul(
                                pav[0:D + 1, :], v_sb[:, mo, h, :],
                                E_pair[:, mo, j, :],
                                start=(mo == 0), stop=(mo == MO - 1))
                        o_un = ou_pool.tile([P, NQ], F16, tag="ou",
                                            name=f"ou{h}")
                        o_uns.append(o_un)
                        nc.vector.tensor_copy(o_un[pb:pb + 64, :], pav[0:D, :])
                        nc.vector.tensor_copy(
                            den_p[32 * j:32 * j + 1, :], pav[D:D + 1, :])
                    rcp_p = sm_pool.tile([33, NQ], F16, tag="rcp",
                                         name=f"rcp{t}")
                    with nc.allow_low_precision(
                            reason="softmax 1/den in fp16 (~5e-4)"):
                        nc.vector.reciprocal(rcp_p[:], den_p[:])
                    for j in range(2):
                        pb = 64 * j
                        pbc = ps_av.tile([64, NQ], F32, tag="ps_av",
                                         name=f"pbc{2 * t + j}")
                        nc.tensor.matmul(
                            pbc[:], ones_q[32 * j:32 * j + 1, :],
                            rcp_p[32 * j:32 * j + 1, :],
                            start=True, stop=True,
                            tile_position=(32 * j, 0))
                        nc.vector.tensor_mul(
                            o_sb[pb:pb + 64, t, :],
                            pbc[:], o_uns[j][pb:pb + 64, :])

                # skewed pipeline: exp of pair 0 starts before the (long) v
                # projection occupies the PE; attention of pair t overlaps
                # k/q projections of pair t+2 and S/exp of pair t+1.
                wk0 = w_pool.tile([P, CO, P], F16, tag="wqk")
                nc.sync.dma_start(wk0[:], wqkv_t[:, :, C:C + P])
                wq0 = w_pool.tile([P, CO, P], F16, tag="wqk")
                nc.sync.dma_start(wq0[:], wqkv_t[:, :, 0:P])
                for co in range(1, CO):
                    nc.sync.dma_start(xT[:, co, :], xbT_t[:, co, :])
                kq_proj(0, wk=wk0, wq=wq0)
                Es = {0: s_exp(0)}
                kq_proj(1)
                Es[1] = s_exp(1)
                kq_proj(2)

                # ---- v projection: [m, e] ----
                for vh in range(2):
                    wv = w_pool.tile([P, CO, NQ], F16, tag="wv", name=f"wv{vh}")
                    nc.sync.dma_start(
                        wv[:], wqkv_t[:, :, 2 * C + vh * NQ:2 * C + (vh + 1) * NQ])
                    for mo in range(MO):
                        pv = ps_proj.tile([P, NQ], F32, tag="pp",
                                          name=f"pv{vh}_{mo}")
                        for co in range(CO):
                            nc.tensor.matmul(
                                pv[:], xT[:, co, mo * P:(mo + 1) * P],
                                wv[:, co, :],
                                start=(co == 0), stop=(co == CO - 1))
                        nc.vector.tensor_scalar_mul(
                            v_sb[:, mo, vh * 8:(vh + 1) * 8, 0:D],
                            pv[:].rearrange("p (h d) -> p h d", d=D),
                            mask_sb[:, mo:mo + 1])
                for mo in range(MO):
                    nc.vector.tensor_scalar_mul(
                        v_sb[:, mo, :, D], onesH[:], mask_sb[:, mo:mo + 1])

                for t in range(H // 2):
                    if t + 2 < H // 2:
                        Es[t + 2] = s_exp(t + 2)
                    if t + 3 < H // 2:
                        kq_proj(t + 3)
                    if t == 5:  # stage w_out late, off the critical DMA path
                        for eo in range(EO):
                            nc.sync.dma_start(wo[:, eo, :], wout_t[:, eo, :])
                    av_norm(t, Es.pop(t))

                # ---- output projection + bias ----
                for no in range(NO):
                    for ch in range(2):
                        py = ps_proj.tile([P, NQ], F32, tag="pp",
                                          name=f"py{no}_{ch}")
                        for eo in range(EO):
                            nc.tensor.matmul(
                                py[:], o_sb[:, eo, no * P:(no + 1) * P],
                                wo[:, eo, ch * NQ:(ch + 1) * NQ],
                                start=(eo == 0), stop=False)
                        nc.tensor.matmul(
                            py[:], ones_row[:],
                            bias_sb[:, ch * NQ:(ch + 1) * NQ],
                            start=False, stop=True)
                        ysb = ou_pool.tile([P, NQ], F32, tag="ysb",
                                           name=f"ysb{no}_{ch}")
                        nc.vector.tensor_copy(ysb[:], py[:])
                        nc.sync.dma_start(
                            y_t[:, no, ch * NQ:(ch + 1) * NQ], ysb[:])

    nc.finalize()
    return nc


_NC_CACHE = None


def _get_nc():
    global _NC_CACHE
    if _NC_CACHE is None:
        _NC_CACHE = build_nc()
    return _NC_CACHE


def _make_in_maps(x, mask, w_qkv, w_out, b_out):
    x = np.ascontiguousarray(np.asarray(x), dtype=np.float32)
    mask_f = np.asarray(mask).astype(np.float32)
    wqkv_h = np.asarray(w_qkv).astype(np.float16)
    wout_h = np.asarray(w_out).astype(np.float16)
    bout_h = np.asarray(b_out).astype(np.float16)
    in_maps = []
    for i in range(N_CORES):
        b, q0 = i // 2, (i % 2) * NQ
        xbT = np.ascontiguousarray(np.roll(x[b], -q0, axis=0).T.astype(np.float16))
        mb = np.ascontiguousarray(np.roll(mask_f[b], -q0))
        in_maps.append({"xbT": xbT, "maskb": mb, "w_qkv": wqkv_h,
                        "w_out": wout_h, "b_out": bout_h})
    return in_maps


def run_kernel(x, mask, w_qkv, w_out, b_out, trace=False):
    """Run on 8 cores; returns (full output [B,N,C], BassKernelResults)."""
    nc = _get_nc()
    in_maps = _make_in_maps(x, mask, w_qkv, w_out, b_out)
    res = run_bass_kernel_spmd(nc, in_maps, core_ids=list(range(N_CORES)),
                               trace=trace)
    out = np.empty((B, N, C), dtype=np.float32)
    for i in range(N_CORES):
        b, q0 = i // 2, (i % 2) * NQ
        out[b, q0:q0 + NQ, :] = res.results[i]["y"]
    return out, res


def kernel(x, mask, w_qkv, w_out, b_out):
    os.environ.setdefault("BASS_NEVER_TRACE", "1")
    out, _ = run_kernel(x, mask, w_qkv, w_out, b_out, trace=False)
    return out


# revision 20
# speedup vs baseline: 1.0730x; 1.0730x over previous
"""Multi-head attention (B=4, N=1024, C=1024, H=16, D=64) on 8 Trainium2 cores.

Sharding: query-parallel, no collectives. Core i handles batch b = i//2 and
query rows (i%2)*512..+512 (the host rolls x[b] so each core's query rows come
first; key order is irrelevant to softmax). Each core computes k/v for its
whole batch, attention for its 512 query rows over all 16 heads, and the
output projection for those rows. The host concatenates the 8 row-slices --
softmax rows are independent, so no cross-core reduction is needed.

Matmuls run in fp16 (1 PE cycle/row vs 1.5 for fp32r, FWL weight loads,
half-size DMA, 10-bit mantissa). Accumulation is always fp32 in PSUM. exp is
computed as exp(s/8 - 12*ln2) so unnormalized attention outputs stay in fp16
range; the 2^-12 factor cancels in the softmax normalization.

Per-core pipeline (x^T is prepared on the host -- input marshalling):
  1. v = x @ w_v -> [m, h, d] with an appended ones column, key mask folded in.
  2. Per head pair t: k^T/q^T column projections for pair t only, then
     S^T = k^T.T @ q^T (K=64 row-tiles, the pair alternates PE row groups
     0/64 so matmuls overlap) -> exp on ACT -> out^T (unnormalized) and the
     softmax denominator via the ones column -> stage to SBUF.
     Attention of pair t overlaps projections of pair t+1 on the PE.
  3. Per head quad: one DVE reciprocal of the 4 denominators, gpsimd
     partition-broadcast, DVE in-place normalize of out^T.
  4. y = o^T.T @ w_out + bias (K=1 ones-row matmul), DMA out.
"""

import os

import numpy as np

import concourse.bacc as bacc
import concourse.mybir as mybir
import concourse.tile as tile
from concourse.bass_utils import run_bass_kernel_spmd

F32 = mybir.dt.float32
F16 = mybir.dt.float16

B, N, C = 4, 1024, 1024
H, D = 16, 64
NQ = 512          # query rows per core
P = 128
MO = N // P       # 8 key m-tiles
CO = C // P       # 8 contraction tiles
EO = C // P       # 8 e-tiles for q (and separately k)
NO = NQ // P      # 4 output row tiles
ATT_SCALE = D ** -0.5
EXP_BIAS = float(-12.0 * np.log(2.0))  # keep out^T in fp16 range
N_CORES = 8


def build_nc():
    nc = bacc.Bacc()
    xbT = nc.declare_dram_parameter("xbT", [C, N], F16, isOutput=False)
    maskb = nc.declare_dram_parameter("maskb", [N], F32, isOutput=False)
    w_qkv = nc.declare_dram_parameter("w_qkv", [C, 3 * H * D], F16, isOutput=False)
    w_out = nc.declare_dram_parameter("w_out", [H * D, C], F16, isOutput=False)
    b_out = nc.declare_dram_parameter("b_out", [C], F16, isOutput=False)
    y = nc.declare_dram_parameter("y", [NQ, C], F32, isOutput=True)

    xbT_t = xbT.rearrange("(co p) m -> p co m", p=P)
    wqkv_t = w_qkv.rearrange("(co p) e -> p co e", p=P)
    wout_t = w_out.rearrange("(eo p) c -> p eo c", p=P)
    y_t = y.rearrange("(no p) c -> p no c", p=P)

    with tile.TileContext(nc) as tc:
        with tc.tile_pool(name="consts", bufs=1) as consts, \
             tc.tile_pool(name="persist", bufs=1) as persist:
            # ---- constants ----
            ones_row = consts.tile([1, P], F16)      # K=1 bias matmul lhsT
            nc.vector.memset(ones_row[:], 1.0)
            onesH = consts.tile([P, H], F16)
            nc.vector.memset(onesH[:], 1.0)
            ones_q = consts.tile([97, 64], F16)
            nc.vector.memset(ones_q[:], 1.0)
            mask_sb = consts.tile([P, MO], F32)
            nc.sync.dma_start(mask_sb[:], maskb.rearrange("(o p) -> p o", p=P))
            ebias = consts.tile([P, 1], F32)
            nc.vector.memset(ebias[:], EXP_BIAS)
            bias_sb = consts.tile([1, C], F16)
            nc.sync.dma_start(bias_sb[:], b_out[None, :])

            # ---- persistent tensors ----
            qT = persist.tile([P, EO, NQ], F16)          # q^T: [e, n]
            kT = persist.tile([P, EO, N], F16)           # k^T: [e, m]
            v_sb = persist.tile([P, MO, H, D + 1], F16)  # v + ones col
            o_sb = persist.tile([P, EO, NQ], F16)        # out^T: [e, n]
            wo = persist.tile([P, EO, C], F16)           # w_out staged early

            with tc.tile_pool(name="xT_pool", bufs=1) as xT_pool, \
                 tc.tile_pool(name="w_pool", bufs=4) as w_pool, \
                 tc.tile_pool(name="E_pool", bufs=3) as E_pool, \
                 tc.tile_pool(name="sm_pool", bufs=2) as sm_pool, \
                 tc.tile_pool(name="ou_pool", bufs=5) as ou_pool, \
                 tc.tile_pool(name="ps_proj", bufs=2, space="PSUM") as ps_proj, \
                 tc.tile_pool(name="ps_s", bufs=2, space="PSUM") as ps_s, \
                 tc.tile_pool(name="ps_av", bufs=2, space="PSUM") as ps_av:
                xT = xT_pool.tile([P, CO, N], F16)       # x^T: [c, m]
                nc.sync.dma_start(xT[:, 0, :], xbT_t[:, 0, :])

                def kq_proj(t, wk=None, wq=None):
                    # k^T columns for pair t (k block starts at e = C)
                    if wk is None:
                        wk = w_pool.tile([P, CO, P], F16, tag="wqk",
                                         name=f"wk{t}")
                        nc.sync.dma_start(
                            wk[:], wqkv_t[:, :, C + t * P:C + (t + 1) * P])
                    for half in range(2):
                        pk = ps_proj.tile([P, NQ], F32, tag="pp", name=f"pk{t}_{half}")
                        for co in range(CO):
                            nc.tensor.matmul(
                                pk[:], wk[:, co, :],
                                xT[:, co, half * NQ:(half + 1) * NQ],
                                start=(co == 0), stop=(co == CO - 1))
                        nc.vector.tensor_copy(
                            kT[:, t, half * NQ:(half + 1) * NQ], pk[:])
                    if wq is None:
                        wq = w_pool.tile([P, CO, P], F16, tag="wqk",
                                         name=f"wq{t}")
                        nc.sync.dma_start(
                            wq[:], wqkv_t[:, :, t * P:(t + 1) * P])
                    pq = ps_proj.tile([P, NQ], F32, tag="pp", name=f"pq{t}")
                    for co in range(CO):
                        nc.tensor.matmul(
                            pq[:], wq[:, co, :], xT[:, co, 0:NQ],
                            start=(co == 0), stop=(co == CO - 1))
                    nc.vector.tensor_copy(qT[:, t, :], pq[:])

                def s_exp(t):
                    # S^T + exp: one [128,1024] psum per mo holds both heads of
                    # the pair (row groups 0/64 -> the two matmuls overlap)
                    E_pair = E_pool.tile([P, MO, 2, NQ], F16, tag="E",
                                         name=f"E{t}")
                    for mo in range(MO):
                        pss = ps_s.tile([P, 2 * NQ], F32, tag="ps_s",
                                        name=f"pss{t}_{mo}")
                        for j in range(2):
                            pb = 64 * j
                            nc.tensor.matmul(
                                pss[:, j * NQ:(j + 1) * NQ],
                                kT[pb:pb + 64, t, mo * P:(mo + 1) * P],
                                qT[pb:pb + 64, t, :],
                                start=True, stop=True)
                        nc.scalar.activation(
                            E_pair[:, mo, :, :], pss[:],
                            mybir.ActivationFunctionType.Exp,
                            bias=ebias[:], scale=ATT_SCALE)
                    return E_pair

                norm_state = {}

                def av_norm(t, E_pair):
                    # AV + stage unnormalized out^T and denominator
                    if t % 2 == 0:
                        den_q = sm_pool.tile([97, NQ], F32, tag="den",
                                             name=f"den{t}")
                        nc.vector.memset(den_q[:], 1.0)
                        norm_state["den"] = den_q
                        norm_state["ou"] = []
                    den_q = norm_state["den"]
                    for j in range(2):
                        h = 2 * t + j
                        pb = 64 * j
                        pav = ps_av.tile([P, NQ], F32, tag="ps_av",
                                         name=f"pav{h}")
                        for mo in range(MO):
                            nc.tensor.matmul(
                                pav[0:D + 1, :], v_sb[:, mo, h, :],
                                E_pair[:, mo, j, :],
                                start=(mo == 0), stop=(mo == MO - 1))
                        o_un = ou_pool.tile([P, NQ], F16, tag="ou",
                                            name=f"ou{h}")
                        norm_state["ou"].append(o_un)
                        nc.vector.tensor_copy(o_un[pb:pb + 64, :], pav[0:D, :])
                        nc.vector.tensor_copy(
                            den_q[32 * (h % 4):32 * (h % 4) + 1, :],
                            pav[D:D + 1, :])
                    # normalize the completed quad
                    if t % 2 == 1:
                        rcp_q = sm_pool.tile([97, NQ], F16, tag="rcp",
                                             name=f"rcp{t}")
                        with nc.allow_low_precision(
                                reason="softmax 1/den in fp16 (~5e-4)"):
                            nc.vector.reciprocal(rcp_q[:], den_q[:])
                        for r in range(4):
                            h = 4 * (t // 2) + r
                            tt, pb = h // 2, 64 * (h % 2)
                            pbc = ps_av.tile([64, NQ], F32, tag="ps_av",
                                             name=f"pbc{h}")
                            nc.tensor.matmul(
                                pbc[:], ones_q[32 * r:32 * r + 1, :],
                                rcp_q[32 * r:32 * r + 1, :],
                                start=True, stop=True,
                                tile_position=(32 * r, 0))
                            nc.vector.tensor_mul(
                                o_sb[pb:pb + 64, tt, :],
                                pbc[:], norm_state["ou"][r][pb:pb + 64, :])

                # skewed pipeline: exp of pair 0 starts before the (long) v
                # projection occupies the PE; attention of pair t overlaps
                # k/q projections of pair t+2 and S/exp of pair t+1.
                wk0 = w_pool.tile([P, CO, P], F16, tag="wqk")
                nc.sync.dma_start(wk0[:], wqkv_t[:, :, C:C + P])
                wq0 = w_pool.tile([P, CO, P], F16, tag="wqk")
                nc.sync.dma_start(wq0[:], wqkv_t[:, :, 0:P])
                for co in range(1, CO):
                    nc.sync.dma_start(xT[:, co, :], xbT_t[:, co, :])
                kq_proj(0, wk=wk0, wq=wq0)
                Es = {0: s_exp(0)}
                kq_proj(1)
                Es[1] = s_exp(1)
                kq_proj(2)

                # ---- v projection: [m, e] ----
                for vh in range(2):
                    wv = w_pool.tile([P, CO, NQ], F16, tag="wv", name=f"wv{vh}")
                    nc.sync.dma_start(
                        wv[:], wqkv_t[:, :, 2 * C + vh * NQ:2 * C + (vh + 1) * NQ])
                    for mo in range(MO):
                        pv = ps_proj.tile([P, NQ], F32, tag="pp",
                                          name=f"pv{vh}_{mo}")
                        for co in range(CO):
                            nc.tensor.matmul(
                                pv[:], xT[:, co, mo * P:(mo + 1) * P],
                                wv[:, co, :],
                                start=(co == 0), stop=(co == CO - 1))
                        nc.vector.tensor_scalar_mul(
                            v_sb[:, mo, vh * 8:(vh + 1) * 8, 0:D],
                            pv[:].rearrange("p (h d) -> p h d", d=D),
                            mask_sb[:, mo:mo + 1])
                for mo in range(MO):
                    nc.vector.tensor_scalar_mul(
                        v_sb[:, mo, :, D], onesH[:], mask_sb[:, mo:mo + 1])

                for t in range(H // 2):
                    if t + 2 < H // 2:
                        Es[t + 2] = s_exp(t + 2)
                    if t + 3 < H // 2:
                        kq_proj(t + 3)
                    if t == 5:  # stage w_out late, off the critical DMA path
                        for eo in range(EO):
                            nc.sync.dma_start(wo[:, eo, :], wout_t[:, eo, :])
                    av_norm(t, Es.pop(t))

                # ---- output projection + bias ----
                for no in range(NO):
                    for ch in range(2):
                        py = ps_proj.tile([P, NQ], F32, tag="pp",
                                          name=f"py{no}_{ch}")
                        for eo in range(EO):
                            nc.tensor.matmul(
                                py[:], o_sb[:, eo, no * P:(no + 1) * P],
                                wo[:, eo, ch * NQ:(ch + 1) * NQ],
                                start=(eo == 0), stop=False)
                        nc.tensor.matmul(
                            py[:], ones_row[:],
                            bias_sb[:, ch * NQ:(ch + 1) * NQ],
                            start=False, stop=True)
                        ysb = ou_pool.tile([P, NQ], F32, tag="ysb",
                                           name=f"ysb{no}_{ch}")
                        nc.vector.tensor_copy(ysb[:], py[:])
                        nc.sync.dma_start(
                            y_t[:, no, ch * NQ:(ch + 1) * NQ], ysb[:])

    nc.finalize()
    return nc


_NC_CACHE = None


def _get_nc():
    global _NC_CACHE
    if _NC_CACHE is None:
        _NC_CACHE = build_nc()
    return _NC_CACHE


def _make_in_maps(x, mask, w_qkv, w_out, b_out):
    x = np.ascontiguousarray(np.asarray(x), dtype=np.float32)
    mask_f = np.asarray(mask).astype(np.float32)
    wqkv_h = np.asarray(w_qkv).astype(np.float16)
    wout_h = np.asarray(w_out).astype(np.float16)
    bout_h = np.asarray(b_out).astype(np.float16)
    in_maps = []
    for i in range(N_CORES):
        b, q0 = i // 2, (i % 2) * NQ
        xbT = np.ascontiguousarray(np.roll(x[b], -q0, axis=0).T.astype(np.float16))
        mb = np.ascontiguousarray(np.roll(mask_f[b], -q0))
        in_maps.append({"xbT": xbT, "maskb": mb, "w_qkv": wqkv_h,
                        "w_out": wout_h, "b_out": bout_h})
    return in_maps


def run_kernel(x, mask, w_qkv, w_out, b_out, trace=False):
    """Run on 8 cores; returns (full output [B,N,C], BassKernelResults)."""
    nc = _get_nc()
    in_maps = _make_in_maps(x, mask, w_qkv, w_out, b_out)
    res = run_bass_kernel_spmd(nc, in_maps, core_ids=list(range(N_CORES)),
                               trace=trace)
    out = np.empty((B, N, C), dtype=np.float32)
    for i in range(N_CORES):
        b, q0 = i // 2, (i % 2) * NQ
        out[b, q0:q0 + NQ, :] = res.results[i]["y"]
    return out, res


def kernel(x, mask, w_qkv, w_out, b_out):
    os.environ.setdefault("BASS_NEVER_TRACE", "1")
    out, _ = run_kernel(x, mask, w_qkv, w_out, b_out, trace=False)
    return out
